# revision 1
# baseline (speedup 1.0000x reference)
"""Trainium2 Bass kernel for nn_MultiHeadAttention (B=2, S=2048, D=1024, H=16).

Sharding: 8 cores = 2 batches x 4 head-groups (4 heads / 256 dims each).
Each core computes its head-group's QKV projections, attention, and a
partial output projection (Megatron row-parallel); host sums the 4
partials per batch and adds the bias terms.

Layouts: host pre-transposes activations/weights so every matmul operand
is DMA-able with the contraction dim on SBUF partitions. All matmuls run
as float32r (TF32-like, 11-bit mantissa, full PE speed); inputs are
pre-rounded on host so device results are deterministic.
"""
import sys
sys.path.insert(0, '/opt/trn_rl_repo')

from contextlib import ExitStack

import numpy as np

import concourse.bass as bass
import concourse.mybir as mybir
import concourse.tile as tile
from concourse import bacc
from concourse.bass_utils import run_bass_kernel_spmd

B, S, D, H = 2, 2048, 1024, 16
HD = D // H            # 64
NCORES = 8
GROUPS = 4             # head groups (tensor parallel)
DL = D // GROUPS       # 256 local d_out per core
HL = H // GROUPS       # 4 local heads
P = 128
KC = S // P            # 16 k-chunks
SC = D // P            # 8 d_in chunks
NSLAB = 4              # s-slabs of 512 for projections
F32R = mybir.dt.float32r
F32 = mybir.dt.float32


def _round_f32r(x):
    """Round fp32 to float32r (11 explicit mantissa bits, round-to-nearest)."""
    xi = np.ascontiguousarray(x, np.float32).view(np.uint32).astype(np.uint64)
    add = np.uint64(1 << 11)
    mask = np.uint64(0xFFFFFFFFFFFFF000)
    return ((xi + add) & mask).astype(np.uint32).view(np.float32)


def _build_module():
    nc = bacc.Bacc(None, target_bir_lowering=False, debug=False)

    qT = nc.dram_tensor("qT", [D, S], F32R, kind="ExternalInput").ap()
    kT = nc.dram_tensor("kT", [D, S], F32R, kind="ExternalInput").ap()
    vT = nc.dram_tensor("vT", [D, S], F32R, kind="ExternalInput").ap()
    wqT = nc.dram_tensor("wqT", [D, DL], F32R, kind="ExternalInput").ap()
    wkT = nc.dram_tensor("wkT", [D, DL], F32R, kind="ExternalInput").ap()
    wvT = nc.dram_tensor("wvT", [D, DL], F32R, kind="ExternalInput").ap()
    woT = nc.dram_tensor("woT", [DL, D], F32R, kind="ExternalInput").ap()
    bq2 = nc.dram_tensor("bq2", [2, P], F32, kind="ExternalInput").ap()
    bk2 = nc.dram_tensor("bk2", [2, P], F32, kind="ExternalInput").ap()
    out = nc.dram_tensor("out", [S, D], F32, kind="ExternalOutput").ap()

    qTv = qT.rearrange("(kc p) s -> p kc s", p=P)
    kTv = kT.rearrange("(kc p) s -> p kc s", p=P)
    vTv = vT.rearrange("(kc p) s -> p kc s", p=P)

    with tile.TileContext(nc) as tc:
        with ExitStack() as ctx:
            wpool = ctx.enter_context(tc.tile_pool(name="weights", bufs=1))
            big = ctx.enter_context(tc.tile_pool(name="big", bufs=1))
            slab = ctx.enter_context(tc.tile_pool(name="slab", bufs=2))
            ptp = ctx.enter_context(tc.tile_pool(name="pt", bufs=4))
            recp = ctx.enter_context(tc.tile_pool(name="rec", bufs=2))
            outp = ctx.enter_context(tc.tile_pool(name="outsb", bufs=2))

            # ---- weights / constants ----
            wq_sb = wpool.tile([P, SC, DL], F32R)
            wk_sb = wpool.tile([P, SC, DL], F32R)
            wv_sb = wpool.tile([P, SC, DL], F32R)
            wo_sb = wpool.tile([P, DL // P, D], F32R)
            bq_sb = wpool.tile([P, 2], F32)
            bk_sb = wpool.tile([P, 2], F32)
            ones_sb = wpool.tile([P, HD], F32)
            nc.sync.dma_start(wq_sb[:], wqT.rearrange("(kc p) m -> p kc m", p=P))
            nc.sync.dma_start(wk_sb[:], wkT.rearrange("(kc p) m -> p kc m", p=P))
            nc.sync.dma_start(wv_sb[:], wvT.rearrange("(kc p) m -> p kc m", p=P))
            nc.sync.dma_start(wo_sb[:], woT.rearrange("(c p) n -> p c n", p=P))
            nc.sync.dma_start(bq_sb[:], bq2.rearrange("m p -> p m"))
            nc.sync.dma_start(bk_sb[:], bk2.rearrange("m p -> p m"))
            nc.gpsimd.memset(ones_sb[:], 1.0)

            # ---- persistent activations ----
            QT = big.tile([P, 2, S], F32R)          # [d_out within pair, m-chunk, q]
            KT = big.tile([P, 2, S], F32R)
            V2 = big.tile([P, KC, HL, 2 * HD], F32R)  # [k, chunk, head, V|ones]
            xT = big.tile([P, 2, S], F32R)          # attention out, transposed

            nc.vector.tensor_copy(
                V2[:, :, :, HD:2 * HD],
                ones_sb[:, None, None, :].to_broadcast([P, KC, HL, HD]),
            )

            # ---- phase 2: projections ----
            with tc.tile_pool(name="proj_ps", bufs=2, space="PSUM") as proj_ps, \
                 tc.tile_pool(name="projv_ps", bufs=2, space="PSUM") as projv_ps:
                for j in range(NSLAB):
                    js = slice(j * 512, (j + 1) * 512)
                    qslab = slab.tile([P, SC, 512], F32R, tag="slab")
                    nc.sync.dma_start(qslab[:, 0:2, :], qTv[:, 0:2, js])
                    nc.sync.dma_start(qslab[:, 2:4, :], qTv[:, 2:4, js])
                    nc.sync.dma_start(qslab[:, 4:6, :], qTv[:, 4:6, js])
                    nc.sync.dma_start(qslab[:, 6:8, :], qTv[:, 6:8, js])
                    for m in range(2):
                        ps = proj_ps.tile([P, 512], F32)
                        for kc in range(SC):
                            nc.tensor.matmul(
                                ps[:], wq_sb[:, kc, m * P:(m + 1) * P],
                                qslab[:, kc, :],
                                start=(kc == 0), stop=(kc == SC - 1))
                        nc.vector.tensor_scalar_add(
                            QT[:, m, js], ps[:], bq_sb[:, m:m + 1])

                    kslab = slab.tile([P, SC, 512], F32R, tag="slab")
                    nc.sync.dma_start(kslab[:, 0:2, :], kTv[:, 0:2, js])
                    nc.sync.dma_start(kslab[:, 2:4, :], kTv[:, 2:4, js])
                    nc.sync.dma_start(kslab[:, 4:6, :], kTv[:, 4:6, js])
                    nc.sync.dma_start(kslab[:, 6:8, :], kTv[:, 6:8, js])
                    for m in range(2):
                        ps = proj_ps.tile([P, 512], F32)
                        for kc in range(SC):
                            nc.tensor.matmul(
                                ps[:], wk_sb[:, kc, m * P:(m + 1) * P],
                                kslab[:, kc, :],
                                start=(kc == 0), stop=(kc == SC - 1))
                        nc.vector.tensor_scalar_add(
                            KT[:, m, js], ps[:], bk_sb[:, m:m + 1])

                    vslab = slab.tile([P, SC, 512], F32R, tag="slab")
                    nc.sync.dma_start(vslab[:, 0:2, :], vTv[:, 0:2, js])
                    nc.sync.dma_start(vslab[:, 2:4, :], vTv[:, 2:4, js])
                    nc.sync.dma_start(vslab[:, 4:6, :], vTv[:, 4:6, js])
                    nc.sync.dma_start(vslab[:, 6:8, :], vTv[:, 6:8, js])
                    for ss in range(4):
                        psv = projv_ps.tile([P, DL], F32)
                        for kc in range(SC):
                            nc.tensor.matmul(
                                psv[:], vslab[:, kc, ss * P:(ss + 1) * P],
                                wv_sb[:, kc, :],
                                start=(kc == 0), stop=(kc == SC - 1))
                        nc.vector.tensor_copy(
                            V2[:, j * 4 + ss, :, 0:HD],
                            psv[:].rearrange("p (h d) -> p h d", d=HD))

            # ---- phase 3: attention per head ----
            with tc.tile_pool(name="st_ps", bufs=2, space="PSUM") as st_ps, \
                 tc.tile_pool(name="av_ps", bufs=1, space="PSUM") as av_ps:
                for h in range(HL):
                    hp, hm = (h % 2) * HD, h // 2
                    av = av_ps.tile([P, S], F32, tag="av")

                    def emit_av(kc, half, pt):
                        for qq in range(2):
                            q0 = half * 1024 + qq * 512
                            nc.tensor.matmul(
                                av[:, q0:q0 + 512],
                                V2[:, kc, h, :], pt[:, qq * 512:(qq + 1) * 512],
                                start=(kc == 0), stop=(kc == KC - 1))

                    pending = None  # one-deep SW pipeline: PE never waits on ACT
                    for kc in range(KC):
                        for half in range(2):
                            st = st_ps.tile([P, 1024], F32, tag="st")
                            for qq in range(2):
                                q0 = half * 1024 + qq * 512
                                nc.tensor.matmul(
                                    st[:, qq * 512:(qq + 1) * 512],
                                    KT[hp:hp + HD, hm, kc * P:(kc + 1) * P],
                                    QT[hp:hp + HD, hm, q0:q0 + 512],
                                    start=True, stop=True)
                            pt = ptp.tile([P, 1024], F32R, tag="pt")
                            nc.scalar.activation(
                                pt[:], st[:],
                                mybir.ActivationFunctionType.Exp, scale=0.125)
                            if pending is not None:
                                emit_av(*pending)
                            pending = (kc, half, pt)
                    emit_av(*pending)
                    rec = recp.tile([HD, S], F32, tag="rec")
                    nc.vector.reciprocal(rec[:], av[HD:2 * HD, :])
                    nc.vector.tensor_tensor(
                        xT[hp:hp + HD, hm, :], av[0:HD, :], rec[:],
                        mybir.AluOpType.mult)

            # ---- phase 4: output projection (partial) ----
            with tc.tile_pool(name="op_ps", bufs=2, space="PSUM") as op_ps:
                for j in range(KC):
                    op = op_ps.tile([P, 1024], F32)
                    for n in range(2):
                        for ci in range(2):
                            nc.tensor.matmul(
                                op[:, n * 512:(n + 1) * 512],
                                xT[:, ci, j * P:(j + 1) * P],
                                wo_sb[:, ci, n * 512:(n + 1) * 512],
                                start=(ci == 0), stop=(ci == 1))
                    osb = outp.tile([P, 1024], F32, tag="osb")
                    nc.vector.tensor_copy(osb[:], op[:])
                    nc.sync.dma_start(out[j * P:(j + 1) * P, :], osb[:])

    nc.compile()
    return nc


_NC = None


def _get_nc():
    global _NC
    if _NC is None:
        _NC = _build_module()
    return _NC


def kernel(query, key, value, mask, Wq, bq, Wk, bk, Wv, bv, Wo, bo,
           _trace=False):
    query = np.asarray(query, np.float32)
    key = np.asarray(key, np.float32)
    value = np.asarray(value, np.float32)
    Wq, Wk, Wv, Wo = (np.asarray(w, np.float32) for w in (Wq, Wk, Wv, Wo))
    bq, bk, bv, bo = (np.asarray(b_, np.float32) for b_ in (bq, bk, bv, bo))
    mask = np.asarray(mask, bool)

    # host-side layout prep (shared across the 4 cores of each batch)
    qT = [_round_f32r(query[b].T) for b in range(B)]
    kTh = [_round_f32r(key[b].T) for b in range(B)]
    vTh = [_round_f32r(value[b].T) for b in range(B)]

    in_maps = []
    for c in range(NCORES):
        b, g = c // GROUPS, c % GROUPS
        gs = slice(g * DL, (g + 1) * DL)
        in_maps.append({
            "qT": qT[b], "kT": kTh[b], "vT": vTh[b],
            "wqT": _round_f32r(Wq[gs, :].T),
            "wkT": _round_f32r(Wk[gs, :].T),
            "wvT": _round_f32r(Wv[gs, :].T),
            "woT": _round_f32r(Wo[:, gs].T),
            "bq2": np.ascontiguousarray(bq[gs].reshape(2, P)),
            "bk2": np.ascontiguousarray(bk[gs].reshape(2, P)),
        })

    nc = _get_nc()
    res = run_bass_kernel_spmd(nc, in_maps, core_ids=list(range(NCORES)),
                               trace=_trace)

    extra = (bv @ Wo.T + bo).astype(np.float32)  # bv folds through out-proj
    output = np.zeros((B, S, D), np.float32)
    for c in range(NCORES):
        output[c // GROUPS] += res.results[c]["out"]
    output += extra

    # masked query rows attend uniformly (softmax of constant -1e9)
    if mask.any():
        for b in range(B):
            rows = np.nonzero(mask[b, 0])[0]
            if rows.size:
                v_full = value[b] @ Wv.T + bv
                out_row = v_full.mean(0) @ Wo.T + bo
                output[b, rows, :] = out_row

    if _trace:
        return output, res
    return output



# revision 18
# speedup vs baseline: 1.2469x; 1.2469x over previous
"""Trainium2 Bass kernel for nn_MultiHeadAttention (B=2, S=2048, D=1024, H=16).

Sharding: 8 cores = 2 batches x 4 head-groups (4 heads / 256 dims each).
Each core computes its head-group's QKV projections, attention, and a
partial output projection (Megatron row-parallel); host sums the 4
partials per batch and adds the bias terms.

All operands are fp16: halves HBM traffic vs fp32 and runs every matmul
at 1 cycle/row regardless of moving-dim width. Attention uses a q-major
AV layout (out [q,dv] + separate denominator column), and x[q,dv] is
flipped to x^T[dv,q] with DMA-engine xbar transposes.

Engines execute their queues in order, so emission order is the
schedule: DMAs are priority-ordered (wq,q01,wk,k0,wv,v0,k1,...) and the
deferred projection chunks / transposes / early out-proj chunks are
hooked INTO the attention kc-loops right where their inputs land, so the
Activation engine (exp, the window floor at ~133us) starts ~13us in and
never waits long on PE.
"""
import sys
sys.path.insert(0, '/opt/trn_rl_repo')

from contextlib import ExitStack

import numpy as np

import concourse.bass as bass
import concourse.mybir as mybir
import concourse.tile as tile
from concourse import bacc
from concourse.bass_utils import run_bass_kernel_spmd

B, S, D, H = 2, 2048, 1024, 16
HD = D // H            # 64
NCORES = 8
GROUPS = 4             # head groups (tensor parallel)
DL = D // GROUPS       # 256 local d_out per core
HL = H // GROUPS       # 4 local heads
P = 128
KC = S // P            # 16 k-chunks
SC = D // P            # 8 d_in chunks
F16 = mybir.dt.float16
F32 = mybir.dt.float32


def _build_module():
    nc = bacc.Bacc(None, target_bir_lowering=False, debug=False)

    qT = nc.dram_tensor("qT", [D, S], F16, kind="ExternalInput").ap()
    kT = nc.dram_tensor("kT", [D, S], F16, kind="ExternalInput").ap()
    vT = nc.dram_tensor("vT", [D, S], F16, kind="ExternalInput").ap()
    wqT = nc.dram_tensor("wqT", [D, DL], F16, kind="ExternalInput").ap()
    wkT = nc.dram_tensor("wkT", [D, DL], F16, kind="ExternalInput").ap()
    wvT = nc.dram_tensor("wvT", [D, DL], F16, kind="ExternalInput").ap()
    woT = nc.dram_tensor("woT", [DL, D], F16, kind="ExternalInput").ap()
    bq2 = nc.dram_tensor("bq2", [2, P], F32, kind="ExternalInput").ap()
    bk2 = nc.dram_tensor("bk2", [2, P], F32, kind="ExternalInput").ap()
    out = nc.dram_tensor("out", [S, D], F16, kind="ExternalOutput").ap()

    qTv = qT.rearrange("(kc p) s -> p kc s", p=P)
    kTv = kT.rearrange("(kc p) s -> p kc s", p=P)
    vTv = vT.rearrange("(kc p) s -> p kc s", p=P)

    with tile.TileContext(nc) as tc:
        with ExitStack() as ctx:
            wpool = ctx.enter_context(tc.tile_pool(name="weights", bufs=1))
            big = ctx.enter_context(tc.tile_pool(name="big", bufs=1))
            # q/k slabs live until their deferred m=1 projections deep in
            # the attention stream -> all four stay resident; v slabs are
            # consumed early.
            qslab = ctx.enter_context(tc.tile_pool(name="qslab", bufs=4))
            kslab = ctx.enter_context(tc.tile_pool(name="kslab", bufs=4))
            vslab = ctx.enter_context(tc.tile_pool(name="vslab", bufs=3))
            ptp = ctx.enter_context(tc.tile_pool(name="pt", bufs=10))
            recp = ctx.enter_context(tc.tile_pool(name="rec", bufs=2))
            outp = ctx.enter_context(tc.tile_pool(name="outsb", bufs=2))

            # ---- persistent SBUF ----
            wq_sb = wpool.tile([P, SC, DL], F16)
            wk_sb = wpool.tile([P, SC, DL], F16)
            wv_sb = wpool.tile([P, SC, DL], F16)
            wo_sb = wpool.tile([P, DL // P, D], F16)
            bq_sb = wpool.tile([P, 2], F32)
            bk_sb = wpool.tile([P, 2], F32)
            QT = big.tile([P, 2, S], F16)           # [d_out in pair, m, q]
            KT = big.tile([P, 2, S], F16)
            V2 = big.tile([P, KC, HL, HD + 1], F16)  # [k, kc, head, V|one]
            xq0 = big.tile([P, KC, 2 * HD], F16)    # [q, qc, dv pair0]
            xq1 = big.tile([P, KC, 2 * HD], F16)
            xq = [xq0, xq1]
            xT = big.tile([P, 2, S], F16)           # [dv in pair, ci, q]

            nc.gpsimd.memset(V2[:, :, :, HD:HD + 1], 1.0)

            # ---- PSUM pools (8 banks exactly; closed before out-proj) ----
            psum_ctx = ExitStack()
            st_ps = psum_ctx.enter_context(
                tc.tile_pool(name="st_ps", bufs=2, space="PSUM"))   # 4 banks
            pv_ps = psum_ctx.enter_context(
                tc.tile_pool(name="pv_ps", bufs=2, space="PSUM"))   # 2 banks
            av_ps = psum_ctx.enter_context(
                tc.tile_pool(name="av_ps", bufs=1, space="PSUM"))   # 1 bank
            den_ps = psum_ctx.enter_context(
                tc.tile_pool(name="den_ps", bufs=1, space="PSUM"))  # 1 bank

            # ---- emission helpers (emission order == engine order) ----
            qk_slabs = {}

            def load(kind, view, j):
                pool = {"q": qslab, "k": kslab, "v": vslab}[kind]
                t = pool.tile([P, SC, 512], F16, tag=kind, name="slab_t")
                nc.sync.dma_start(t[:], view[:, :, j * 512:(j + 1) * 512])
                qk_slabs[(kind, j)] = t

            def proj_qk(kind, j, m):
                """m-chunk (128 d_out) of Q/K projection for slab j (512 seq
                positions) -> QT/KT fp16. Uses half of a rotating st tile."""
                t = qk_slabs[(kind, j)]
                w_sb, b_sb, dst = ((wq_sb, bq_sb, QT) if kind == "q"
                                   else (wk_sb, bk_sb, KT))
                ps = st_ps.tile([P, 1024], F32, tag="st", name="ps")
                for kc in range(SC):
                    nc.tensor.matmul(
                        ps[:, 0:512], w_sb[:, kc, m * P:(m + 1) * P],
                        t[:, kc, :],
                        start=(kc == 0), stop=(kc == SC - 1))
                nc.vector.tensor_scalar_add(
                    dst[:, m, j * 512:(j + 1) * 512], ps[:, 0:512],
                    b_sb[:, m:m + 1])

            def proj_v(j, ss):
                """One 128-row chunk (kc index 4j+ss) of the V projection."""
                t = qk_slabs[("v", j)]
                psv = pv_ps.tile([P, DL], F32, tag="pv", name="psv")
                for kc in range(SC):
                    nc.tensor.matmul(
                        psv[:], t[:, kc, ss * P:(ss + 1) * P],
                        wv_sb[:, kc, :],
                        start=(kc == 0), stop=(kc == SC - 1))
                nc.vector.tensor_copy(
                    V2[:, j * 4 + ss, :, 0:HD],
                    psv[:].rearrange("p (h d) -> p h d", d=HD))

            def out_proj(j):
                """Output-projection chunk for query rows [128j, 128j+128)."""
                op = op_ps_pool.tile([P, 1024], F32, tag="op", name="op")
                for n in range(2):
                    for ci in range(2):
                        nc.tensor.matmul(
                            op[:, n * 512:(n + 1) * 512],
                            xT[:, ci, j * P:(j + 1) * P],
                            wo_sb[:, ci, n * 512:(n + 1) * 512],
                            start=(ci == 0), stop=(ci == 1))
                osb = outp.tile([P, 1024], F16, tag="osb", name="osb")
                if j % 2 == 0:
                    nc.scalar.copy(osb[:], op[:])
                else:
                    nc.vector.tensor_copy(osb[:], op[:])
                nc.sync.dma_start(out[j * P:(j + 1) * P, :], osb[:])

            def transpose_half(pair, qh):
                for qc in range(qh * 8, qh * 8 + 8):
                    nc.sync.dma_start_transpose(
                        xT[:, pair, qc * P:(qc + 1) * P],
                        xq[pair][:, qc, :])

            def attention(h, qh, hooks):
                """One (head, q-half) pass; hooks[kc] emits deferred work."""
                hp, hm = (h % 2) * HD, h // 2
                pair = h // 2
                av = av_ps.tile([P, 8, HD], F32, tag="av", name="av")
                den = den_ps.tile([P, 8, 1], F32, tag="den", name="den")

                def emit_av(kc, pt):
                    # One start/stop per PSUM bank: start pends the whole
                    # 2KB zero region; qc>0 first-writes zero-fill it.
                    for qc in range(8):
                        pc = pt[:, qc * P:(qc + 1) * P]
                        nc.tensor.matmul(
                            av[:, qc, :], pc, V2[:, kc, h, 0:HD],
                            start=(kc == 0 and qc == 0),
                            stop=(kc == KC - 1 and qc == 7),
                            skip_group_check=True)
                        nc.tensor.matmul(
                            den[:, qc, :], pc, V2[:, kc, h, HD:HD + 1],
                            start=(kc == 0 and qc == 0),
                            stop=(kc == KC - 1 and qc == 7),
                            skip_group_check=True)

                pending = None  # one-deep pipeline: PE never waits on ACT
                for kc in range(KC):
                    for hook in hooks.get(kc, ()):
                        hook()
                    st = st_ps.tile([P, 1024], F32, tag="st", name="st")
                    for qq in range(2):
                        q0 = qh * 1024 + qq * 512
                        nc.tensor.matmul(
                            st[:, qq * 512:(qq + 1) * 512],
                            KT[hp:hp + HD, hm, kc * P:(kc + 1) * P],
                            QT[hp:hp + HD, hm, q0:q0 + 512],
                            start=True, stop=True)
                    pt = ptp.tile([P, 1024], F16, tag="pt", name="pt")
                    nc.scalar.activation(
                        pt[:], st[:],
                        mybir.ActivationFunctionType.Exp, scale=0.125)
                    if pending is not None:
                        emit_av(*pending)
                    pending = (kc, pt)
                for hook in hooks.get(KC, ()):
                    hook()
                emit_av(*pending)

                rec = recp.tile([P, 8], F32, tag="rec", name="rec")
                nc.vector.reciprocal(rec[:], den[:, :, 0])
                nc.vector.tensor_tensor(
                    xq[pair][:, qh * 8:(qh + 1) * 8, hp:hp + HD],
                    av[:],
                    rec[:, :, None].to_broadcast([P, 8, HD]),
                    mybir.AluOpType.mult)

            # ---- prologue: all input DMAs in priority order + first projs --
            nc.sync.dma_start(wq_sb[:], wqT.rearrange("(kc p) m -> p kc m", p=P))
            nc.sync.dma_start(bq_sb[:], bq2.rearrange("m p -> p m"))
            nc.sync.dma_start(bk_sb[:], bk2.rearrange("m p -> p m"))
            load("q", qTv, 0)
            proj_qk("q", 0, 0)
            load("q", qTv, 1)
            proj_qk("q", 1, 0)
            nc.sync.dma_start(wk_sb[:], wkT.rearrange("(kc p) m -> p kc m", p=P))
            load("k", kTv, 0)
            proj_qk("k", 0, 0)
            nc.sync.dma_start(wv_sb[:], wvT.rearrange("(kc p) m -> p kc m", p=P))
            load("v", vTv, 0)
            load("k", kTv, 1)
            load("v", vTv, 1)
            load("k", kTv, 2)
            load("v", vTv, 2)
            load("k", kTv, 3)
            load("v", vTv, 3)
            load("q", qTv, 2)
            load("q", qTv, 3)
            nc.sync.dma_start(wo_sb[:], woT.rearrange("(c p) n -> p c n", p=P))

            op_ps_pool = None  # opened after pv closes (bank budget)

            # ---- attention sections, qh-outer ----
            # (qh0,h0): k m0 + all V-proj chunks stream in.
            hooks = {4: [lambda: proj_qk("k", 1, 0)],
                     8: [lambda: proj_qk("k", 2, 0)],
                     12: [lambda: proj_qk("k", 3, 0)]}
            for c in range(KC):
                hooks.setdefault(c + 1, []).append(
                    lambda j=c // 4, ss=c % 4: proj_v(j, ss))
            attention(0, 0, hooks)

            # (qh0,h1): spread q m1 projections into the slack.
            attention(1, 0, {4: [lambda: proj_qk("q", 0, 1)],
                             12: [lambda: proj_qk("q", 1, 1)]})
            transpose_half(0, 0)

            # (qh0,h2): k m1 chunks ahead of their score columns.
            attention(2, 0, {0: [lambda: proj_qk("k", 0, 1)],
                             4: [lambda: proj_qk("k", 1, 1)],
                             8: [lambda: proj_qk("k", 2, 1)],
                             12: [lambda: proj_qk("k", 3, 1)]})
            # (qh0,h3): q2/q3 m0 for the qh1 sections.
            attention(3, 0, {4: [lambda: proj_qk("q", 2, 0)],
                             12: [lambda: proj_qk("q", 3, 0)]})
            transpose_half(1, 0)

            attention(0, 1, {4: [lambda: proj_qk("q", 2, 1)]})
            attention(1, 1, {4: [lambda: proj_qk("q", 3, 1)]})
            transpose_half(0, 1)

            attention(2, 1, {})
            attention(3, 1, {})
            transpose_half(1, 1)

            # ---- output projection ----
            psum_ctx.close()  # free the 8 attention banks
            with tc.tile_pool(name="op_ps", bufs=2, space="PSUM") as op_ps:
                op_ps_pool = op_ps
                for j in range(KC):
                    out_proj(j)

    nc.compile()
    return nc


_NC = None


def _get_nc():
    global _NC
    if _NC is None:
        _NC = _build_module()
    return _NC


def kernel(query, key, value, mask, Wq, bq, Wk, bk, Wv, bv, Wo, bo,
           _trace=False):
    query = np.asarray(query, np.float32)
    key = np.asarray(key, np.float32)
    value = np.asarray(value, np.float32)
    Wq, Wk, Wv, Wo = (np.asarray(w, np.float32) for w in (Wq, Wk, Wv, Wo))
    bq, bk, bv, bo = (np.asarray(b_, np.float32) for b_ in (bq, bk, bv, bo))
    mask = np.asarray(mask, bool)

    f16 = lambda x: np.ascontiguousarray(x, np.float16)
    qT = [f16(query[b].T) for b in range(B)]
    kTh = [f16(key[b].T) for b in range(B)]
    vTh = [f16(value[b].T) for b in range(B)]

    in_maps = []
    for c in range(NCORES):
        b, g = c // GROUPS, c % GROUPS
        gs = slice(g * DL, (g + 1) * DL)
        in_maps.append({
            "qT": qT[b], "kT": kTh[b], "vT": vTh[b],
            "wqT": f16(Wq[gs, :].T),
            "wkT": f16(Wk[gs, :].T),
            "wvT": f16(Wv[gs, :].T),
            "woT": f16(Wo[:, gs].T),
            "bq2": np.ascontiguousarray(bq[gs].reshape(2, P)),
            "bk2": np.ascontiguousarray(bk[gs].reshape(2, P)),
        })

    nc = _get_nc()
    res = run_bass_kernel_spmd(nc, in_maps, core_ids=list(range(NCORES)),
                               trace=_trace)

    extra = (bv @ Wo.T + bo).astype(np.float32)  # bv folds through out-proj
    output = np.zeros((B, S, D), np.float32)
    for c in range(NCORES):
        output[c // GROUPS] += res.results[c]["out"].astype(np.float32)
    output += extra

    # masked query rows attend uniformly (softmax of constant -1e9)
    if mask.any():
        for b in range(B):
            rows = np.nonzero(mask[b, 0])[0]
            if rows.size:
                v_full = value[b] @ Wv.T + bv
                out_row = v_full.mean(0) @ Wo.T + bo
                output[b, rows, :] = out_row

    if _trace:
        return output, res
    return output


# revision 21
# speedup vs baseline: 1.4405x; 1.1553x over previous
"""Trainium2 Bass kernel for nn_MultiHeadAttention (B=2, S=2048, D=1024, H=16).

Sharding: 8 cores = 2 batches x 4 head-groups (4 heads / 256 dims each).
Each core computes its head-group's QKV projections, attention, and a
partial output projection (Megatron row-parallel); host sums the 4
partials per batch and adds the bias terms.

All operands are fp16: halves HBM traffic vs fp32 and runs every matmul
at 1 cycle/row regardless of moving-dim width. Attention uses a q-major
AV layout (out [q,dv] + separate denominator column); x[q,dv] is flipped
to x^T[dv,q] with DMA-engine xbar transposes.

Engines execute their queues in order, so emission order is the
schedule. The Activation engine's 128 exp tiles (~133us) are the window
floor: DMAs are priority-ordered so the first exp starts ~20us in, and
all deferred work (m=1 projection chunks, per-head V projections, the
final transposes) is hooked into attention kc-loops sized to each
section's PE slack so exp never waits long.
"""
import sys
sys.path.insert(0, '/opt/trn_rl_repo')

from contextlib import ExitStack

import numpy as np

import concourse.bass as bass
import concourse.mybir as mybir
import concourse.tile as tile
from concourse import bacc
from concourse.bass_utils import run_bass_kernel_spmd

B, S, D, H = 2, 2048, 1024, 16
HD = D // H            # 64
NCORES = 8
GROUPS = 4             # head groups (tensor parallel)
DL = D // GROUPS       # 256 local d_out per core
HL = H // GROUPS       # 4 local heads
P = 128
KC = S // P            # 16 k-chunks
SC = D // P            # 8 d_in chunks
F16 = mybir.dt.float16
F32 = mybir.dt.float32


def _build_module():
    nc = bacc.Bacc(None, target_bir_lowering=False, debug=False)

    qT = nc.dram_tensor("qT", [D, S], F16, kind="ExternalInput").ap()
    kT = nc.dram_tensor("kT", [D, S], F16, kind="ExternalInput").ap()
    vT = nc.dram_tensor("vT", [D, S], F16, kind="ExternalInput").ap()
    wqT = nc.dram_tensor("wqT", [D, DL], F16, kind="ExternalInput").ap()
    wkT = nc.dram_tensor("wkT", [D, DL], F16, kind="ExternalInput").ap()
    wvT = nc.dram_tensor("wvT", [D, DL], F16, kind="ExternalInput").ap()
    woT = nc.dram_tensor("woT", [DL, D], F16, kind="ExternalInput").ap()
    bq2 = nc.dram_tensor("bq2", [2, P], F32, kind="ExternalInput").ap()
    bk2 = nc.dram_tensor("bk2", [2, P], F32, kind="ExternalInput").ap()
    out = nc.dram_tensor("out", [S, D], F16, kind="ExternalOutput").ap()

    qTv = qT.rearrange("(kc p) s -> p kc s", p=P)
    kTv = kT.rearrange("(kc p) s -> p kc s", p=P)
    vTv = vT.rearrange("(kc p) s -> p kc s", p=P)
    outv = out.rearrange("(g c p) n -> g p c n", p=P, c=4)  # 4-row-chunk groups

    with tile.TileContext(nc) as tc:
        with ExitStack() as ctx:
            wpool = ctx.enter_context(tc.tile_pool(name="weights", bufs=1))
            big = ctx.enter_context(tc.tile_pool(name="big", bufs=1))
            qslab = ctx.enter_context(tc.tile_pool(name="qslab", bufs=4))
            kslab = ctx.enter_context(tc.tile_pool(name="kslab", bufs=4))
            # all four v slabs stay resident: per-head V-proj chunks read
            # them across sections 0-3
            vslab = ctx.enter_context(tc.tile_pool(name="vslab", bufs=4))
            ptp = ctx.enter_context(tc.tile_pool(name="pt", bufs=8))
            recp = ctx.enter_context(tc.tile_pool(name="rec", bufs=2))
            outp = ctx.enter_context(tc.tile_pool(name="outsb", bufs=2))

            # ---- persistent SBUF ----
            wq_sb = wpool.tile([P, SC, DL], F16)
            wk_sb = wpool.tile([P, SC, DL], F16)
            wv_sb = wpool.tile([P, SC, DL], F16)
            wo_sb = wpool.tile([P, DL // P, D], F16)
            bq_sb = wpool.tile([P, 2], F32)
            bk_sb = wpool.tile([P, 2], F32)
            QT = big.tile([P, 2, S], F16)           # [d_out in pair, m, q]
            KT = big.tile([P, 2, S], F16)
            V2 = big.tile([P, KC, HL, HD + 1], F16)  # [k, kc, head, V|one]
            xq0 = big.tile([P, KC, 2 * HD], F16)    # [q, qc, dv pair0]
            xq1 = big.tile([P, KC, 2 * HD], F16)
            xq = [xq0, xq1]
            xT = big.tile([P, 2, S], F16)           # [dv in pair, ci, q]

            nc.gpsimd.memset(V2[:, :, :, HD:HD + 1], 1.0)

            # ---- PSUM pools: stA = scores (4 banks), stB = the rest (4) ----
            stA = ExitStack()
            st_ps = stA.enter_context(
                tc.tile_pool(name="st_ps", bufs=2, space="PSUM"))   # 4 banks
            stB = ExitStack()
            qk_ps = stB.enter_context(
                tc.tile_pool(name="qk_ps", bufs=1, space="PSUM"))   # 1 bank
            pv_ps = stB.enter_context(
                tc.tile_pool(name="pv_ps", bufs=1, space="PSUM"))   # 1 bank
            av_ps = stB.enter_context(
                tc.tile_pool(name="av_ps", bufs=1, space="PSUM"))   # 1 bank
            den_ps = stB.enter_context(
                tc.tile_pool(name="den_ps", bufs=1, space="PSUM"))  # 1 bank

            # ---- emission helpers (emission order == engine order) ----
            slabs = {}

            def load(kind, view, j):
                pool = {"q": qslab, "k": kslab, "v": vslab}[kind]
                t = pool.tile([P, SC, 512], F16, tag=kind, name="slab_t")
                nc.sync.dma_start(t[:], view[:, :, j * 512:(j + 1) * 512])
                slabs[(kind, j)] = t

            def proj_qk(kind, j, m):
                """m-chunk (128 d_out) of Q/K projection for slab j."""
                t = slabs[(kind, j)]
                w_sb, b_sb, dst = ((wq_sb, bq_sb, QT) if kind == "q"
                                   else (wk_sb, bk_sb, KT))
                ps = qk_ps.tile([P, 512], F32, tag="qk", name="ps")
                for kc in range(SC):
                    nc.tensor.matmul(
                        ps[:], w_sb[:, kc, m * P:(m + 1) * P], t[:, kc, :],
                        start=(kc == 0), stop=(kc == SC - 1))
                nc.vector.tensor_scalar_add(
                    dst[:, m, j * 512:(j + 1) * 512], ps[:],
                    b_sb[:, m:m + 1])

            def proj_v(c, h):
                """V2[:, c, h, :]: head h's V columns for k-chunk c."""
                t = slabs[("v", c // 4)]
                ss = c % 4
                psv = pv_ps.tile([P, HD], F32, tag="pv", name="psv")
                for kc in range(SC):
                    nc.tensor.matmul(
                        psv[:], t[:, kc, ss * P:(ss + 1) * P],
                        wv_sb[:, kc, h * HD:(h + 1) * HD],
                        start=(kc == 0), stop=(kc == SC - 1))
                nc.vector.tensor_copy(V2[:, c, h, 0:HD], psv[:])

            def out_proj(j, group_dma):
                """Output-projection chunk for query rows [128j, 128j+128)."""
                op = op_holder[0].tile([P, 1024], F32, tag="op", name="op")
                for n in range(2):
                    for ci in range(2):
                        nc.tensor.matmul(
                            op[:, n * 512:(n + 1) * 512],
                            xT[:, ci, j * P:(j + 1) * P],
                            wo_sb[:, ci, n * 512:(n + 1) * 512],
                            start=(ci == 0), stop=(ci == 1))
                g, c = j // 4, j % 4
                osb = osb_tiles[g % 2]
                if j % 2 == 0:
                    nc.scalar.copy(osb[:, c, :], op[:])
                else:
                    nc.vector.tensor_copy(osb[:, c, :], op[:])
                if group_dma and c == 3:
                    nc.sync.dma_start(outv[g], osb[:])

            def transpose(pair, qc):
                nc.sync.dma_start_transpose(
                    xT[:, pair, qc * P:(qc + 1) * P], xq[pair][:, qc, :])

            def attention(h, qh, hooks):
                """One (head, q-half) pass; hooks[kc] emits deferred work."""
                hp, hm = (h % 2) * HD, h // 2
                pair = h // 2
                av = av_ps.tile([P, 8, HD], F32, tag="av", name="av")
                den = den_ps.tile([P, 8, 1], F32, tag="den", name="den")

                def emit_av(kc, pt):
                    # One start/stop per PSUM bank: start pends the whole
                    # 2KB zero region; later qc first-writes zero-fill it.
                    for qc in range(8):
                        pc = pt[:, qc * P:(qc + 1) * P]
                        nc.tensor.matmul(
                            av[:, qc, :], pc, V2[:, kc, h, 0:HD],
                            start=(kc == 0 and qc == 0),
                            stop=(kc == KC - 1 and qc == 7),
                            skip_group_check=True)
                        nc.tensor.matmul(
                            den[:, qc, :], pc, V2[:, kc, h, HD:HD + 1],
                            start=(kc == 0 and qc == 0),
                            stop=(kc == KC - 1 and qc == 7),
                            skip_group_check=True)

                pending = None  # one-deep pipeline: PE never waits on ACT
                for kc in range(KC):
                    for hook in hooks.get(kc, ()):
                        hook()
                    st = st_ps.tile([P, 1024], F32, tag="st", name="st")
                    for qq in range(2):
                        q0 = qh * 1024 + qq * 512
                        nc.tensor.matmul(
                            st[:, qq * 512:(qq + 1) * 512],
                            KT[hp:hp + HD, hm, kc * P:(kc + 1) * P],
                            QT[hp:hp + HD, hm, q0:q0 + 512],
                            start=True, stop=True)
                    pt = ptp.tile([P, 1024], F16, tag="pt", name="pt")
                    nc.scalar.activation(
                        pt[:], st[:],
                        mybir.ActivationFunctionType.Exp, scale=0.125)
                    if pending is not None:
                        emit_av(*pending)
                    pending = (kc, pt)
                for hook in hooks.get(KC, ()):
                    hook()
                emit_av(*pending)

                rec = recp.tile([P, 8], F32, tag="rec", name="rec")
                nc.vector.reciprocal(rec[:], den[:, :, 0])
                nc.vector.tensor_tensor(
                    xq[pair][:, qh * 8:(qh + 1) * 8, hp:hp + HD],
                    av[:],
                    rec[:, :, None].to_broadcast([P, 8, HD]),
                    mybir.AluOpType.mult)

            # ---- prologue: input DMAs in priority order + first projs ----
            nc.sync.dma_start(wq_sb[:], wqT.rearrange("(kc p) m -> p kc m", p=P))
            nc.sync.dma_start(bq_sb[:], bq2.rearrange("m p -> p m"))
            nc.sync.dma_start(bk_sb[:], bk2.rearrange("m p -> p m"))
            load("q", qTv, 0)
            proj_qk("q", 0, 0)
            load("q", qTv, 1)
            proj_qk("q", 1, 0)
            nc.sync.dma_start(wk_sb[:], wkT.rearrange("(kc p) m -> p kc m", p=P))
            load("k", kTv, 0)
            proj_qk("k", 0, 0)
            nc.sync.dma_start(wv_sb[:], wvT.rearrange("(kc p) m -> p kc m", p=P))
            load("v", vTv, 0)
            load("k", kTv, 1)
            load("v", vTv, 1)
            load("k", kTv, 2)
            load("v", vTv, 2)
            load("k", kTv, 3)
            load("v", vTv, 3)
            nc.sync.dma_start(wo_sb[:], woT.rearrange("(c p) n -> p c n", p=P))
            load("q", qTv, 2)
            load("q", qTv, 3)

            # ---- attention sections (qh outer) with balanced hooks ----
            # sec0 (qh0,h0): k m0 slabs + h0's V chunks stream in.
            hooks = {4: [lambda: proj_qk("k", 1, 0)],
                     8: [lambda: proj_qk("k", 2, 0)],
                     12: [lambda: proj_qk("k", 3, 0)]}
            for c in range(KC):
                hooks.setdefault(c + 1, []).append(lambda c=c: proj_v(c, 0))
            attention(0, 0, hooks)

            # sec1 (qh0,h1): h1's V + q m1 + k2/k3 m1 (landed long ago).
            hooks = {2: [lambda: proj_qk("q", 0, 1)],
                     6: [lambda: proj_qk("q", 1, 1)],
                     10: [lambda: proj_qk("k", 2, 1)],
                     14: [lambda: proj_qk("k", 3, 1)]}
            for c in range(KC):
                hooks.setdefault(c + 1, []).append(lambda c=c: proj_v(c, 1))
            attention(1, 0, hooks)
            for qc in range(8):
                transpose(0, qc)

            # sec2 (qh0,h2): h2's V + k0/k1 m1 just ahead of their columns.
            hooks = {0: [lambda: proj_qk("k", 0, 1)],
                     4: [lambda: proj_qk("k", 1, 1)]}
            for c in range(KC):
                hooks.setdefault(c + 1, []).append(lambda c=c: proj_v(c, 2))
            attention(2, 0, hooks)

            # sec3 (qh0,h3): h3's V + q2/q3 m0 for the qh1 half.
            hooks = {4: [lambda: proj_qk("q", 2, 0)],
                     12: [lambda: proj_qk("q", 3, 0)]}
            for c in range(KC):
                hooks.setdefault(c + 1, []).append(lambda c=c: proj_v(c, 3))
            attention(3, 0, hooks)
            for qc in range(8):
                transpose(1, qc)

            attention(0, 1, {4: [lambda: proj_qk("q", 2, 1)]})
            attention(1, 1, {4: [lambda: proj_qk("q", 3, 1)]})
            for qc in range(8, 16):
                transpose(0, qc)

            attention(2, 1, {})
            attention(3, 1, {})

            # ---- tail: free stB's 4 banks for the out-proj accumulators,
            # interleave the last transposes with out-proj chunks ----
            stB.close()
            op_holder = [None]
            osb_tiles = [outp.tile([P, 4, 1024], F16, tag="osb0", name="o0"),
                         outp.tile([P, 4, 1024], F16, tag="osb1", name="o1")]
            with tc.tile_pool(name="op_ps", bufs=2, space="PSUM") as op_ps:
                op_holder[0] = op_ps
                for j in range(8):
                    out_proj(j, True)
                for j in range(8, 16):
                    transpose(1, j)
                    out_proj(j, True)
            stA.close()

    nc.compile()
    return nc


_NC = None


def _get_nc():
    global _NC
    if _NC is None:
        _NC = _build_module()
    return _NC


def kernel(query, key, value, mask, Wq, bq, Wk, bk, Wv, bv, Wo, bo,
           _trace=False):
    query = np.asarray(query, np.float32)
    key = np.asarray(key, np.float32)
    value = np.asarray(value, np.float32)
    Wq, Wk, Wv, Wo = (np.asarray(w, np.float32) for w in (Wq, Wk, Wv, Wo))
    bq, bk, bv, bo = (np.asarray(b_, np.float32) for b_ in (bq, bk, bv, bo))
    mask = np.asarray(mask, bool)

    f16 = lambda x: np.ascontiguousarray(x, np.float16)
    qT = [f16(query[b].T) for b in range(B)]
    kTh = [f16(key[b].T) for b in range(B)]
    vTh = [f16(value[b].T) for b in range(B)]

    in_maps = []
    for c in range(NCORES):
        b, g = c // GROUPS, c % GROUPS
        gs = slice(g * DL, (g + 1) * DL)
        in_maps.append({
            "qT": qT[b], "kT": kTh[b], "vT": vTh[b],
            "wqT": f16(Wq[gs, :].T),
            "wkT": f16(Wk[gs, :].T),
            "wvT": f16(Wv[gs, :].T),
            "woT": f16(Wo[:, gs].T),
            "bq2": np.ascontiguousarray(bq[gs].reshape(2, P)),
            "bk2": np.ascontiguousarray(bk[gs].reshape(2, P)),
        })

    nc = _get_nc()
    res = run_bass_kernel_spmd(nc, in_maps, core_ids=list(range(NCORES)),
                               trace=_trace)

    extra = (bv @ Wo.T + bo).astype(np.float32)  # bv folds through out-proj
    output = np.zeros((B, S, D), np.float32)
    for c in range(NCORES):
        output[c // GROUPS] += res.results[c]["out"].astype(np.float32)
    output += extra

    # masked query rows attend uniformly (softmax of constant -1e9)
    if mask.any():
        for b in range(B):
            rows = np.nonzero(mask[b, 0])[0]
            if rows.size:
                v_full = value[b] @ Wv.T + bv
                out_row = v_full.mean(0) @ Wo.T + bo
                output[b, rows, :] = out_row

    if _trace:
        return output, res
    return output


# revision 24
# speedup vs baseline: 1.5147x; 1.0515x over previous
"""Trainium2 Bass kernel for nn_MultiHeadAttention (B=2, S=2048, D=1024, H=16).

Sharding: 8 cores = 2 batches x 4 head-groups (4 heads / 256 dims each).
Each core computes its head-group's QKV projections, attention, and a
partial output projection (Megatron row-parallel); host sums the 4
partials per batch and adds the bias terms.

All operands are fp16: halves HBM traffic vs fp32 and runs every matmul
at 1 cycle/row regardless of moving-dim width. Attention uses a q-major
AV layout (out [q,dv] + separate denominator column); x[q,dv] is flipped
to x^T[dv,q] with DMA-engine xbar transposes.

Engines execute their queues in order, so emission order is the
schedule. The Activation engine's 128 exp tiles (~133us) are the window
floor. DMAs are priority-ordered so the first exp starts ~17us in; all
deferred work (m=1 projection chunks as 4-matmul half-chains, per-head
V projections, output-projection chunks, transposes) is hooked into the
attention kc-loops sized to each section's PE slack, so exp rarely
waits.
"""
import sys
sys.path.insert(0, '/opt/trn_rl_repo')

from contextlib import ExitStack

import numpy as np

import concourse.bass as bass
import concourse.mybir as mybir
import concourse.tile as tile
from concourse import bacc
from concourse.bass_utils import run_bass_kernel_spmd

B, S, D, H = 2, 2048, 1024, 16
HD = D // H            # 64
NCORES = 8
GROUPS = 4             # head groups (tensor parallel)
DL = D // GROUPS       # 256 local d_out per core
HL = H // GROUPS       # 4 local heads
P = 128
KC = S // P            # 16 k-chunks
SC = D // P            # 8 d_in chunks
F16 = mybir.dt.float16
F32 = mybir.dt.float32


def _build_module():
    nc = bacc.Bacc(None, target_bir_lowering=False, debug=False)

    qT = nc.dram_tensor("qT", [D, S], F16, kind="ExternalInput").ap()
    kT = nc.dram_tensor("kT", [D, S], F16, kind="ExternalInput").ap()
    vT = nc.dram_tensor("vT", [D, S], F16, kind="ExternalInput").ap()
    wqT = nc.dram_tensor("wqT", [D, DL], F16, kind="ExternalInput").ap()
    wkT = nc.dram_tensor("wkT", [D, DL], F16, kind="ExternalInput").ap()
    wvT = nc.dram_tensor("wvT", [D, DL], F16, kind="ExternalInput").ap()
    woT = nc.dram_tensor("woT", [DL, D], F16, kind="ExternalInput").ap()
    bq2 = nc.dram_tensor("bq2", [2, P], F32, kind="ExternalInput").ap()
    bk2 = nc.dram_tensor("bk2", [2, P], F32, kind="ExternalInput").ap()
    out = nc.dram_tensor("out", [S, D], F16, kind="ExternalOutput").ap()

    qTv = qT.rearrange("(kc p) s -> p kc s", p=P)
    kTv = kT.rearrange("(kc p) s -> p kc s", p=P)
    vTv = vT.rearrange("(kc p) s -> p kc s", p=P)
    outv = out.rearrange("(g c p) n -> g p c n", p=P, c=4)  # 4-row-chunk groups

    with tile.TileContext(nc) as tc:
        with ExitStack() as ctx:
            wpool = ctx.enter_context(tc.tile_pool(name="weights", bufs=1))
            big = ctx.enter_context(tc.tile_pool(name="big", bufs=1))
            qslab = ctx.enter_context(tc.tile_pool(name="qslab", bufs=4))
            kslab = ctx.enter_context(tc.tile_pool(name="kslab", bufs=4))
            vslab = ctx.enter_context(tc.tile_pool(name="vslab", bufs=4))
            ptp = ctx.enter_context(tc.tile_pool(name="pt", bufs=8))
            recp = ctx.enter_context(tc.tile_pool(name="rec", bufs=2))
            outp = ctx.enter_context(tc.tile_pool(name="outsb", bufs=1))

            # ---- persistent SBUF ----
            wq_sb = wpool.tile([P, SC, DL], F16)
            wk_sb = wpool.tile([P, SC, DL], F16)
            wv_sb = wpool.tile([P, SC, DL], F16)
            wo_sb = wpool.tile([P, DL // P, D], F16)
            bq_sb = wpool.tile([P, 2], F32)
            bk_sb = wpool.tile([P, 2], F32)
            QT = big.tile([P, 2, S], F16)           # [d_out in pair, m, q]
            KT = big.tile([P, 2, S], F16)
            V2 = big.tile([P, KC, HL, HD + 1], F16)  # [k, kc, head, V|one]
            xq0 = big.tile([P, KC, 2 * HD], F16)    # [q, qc, dv pair0]
            xq1 = big.tile([P, KC, 2 * HD], F16)
            xq = [xq0, xq1]
            xT = big.tile([P, 2, S], F16)           # [dv in pair, ci, q]
            osb_tiles = [outp.tile([P, 4, 1024], F16, tag="osb0", name="o0"),
                         outp.tile([P, 4, 1024], F16, tag="osb1", name="o1")]

            nc.gpsimd.memset(V2[:, :, :, HD:HD + 1], 1.0)

            # ---- PSUM: stA = score tiles (4 banks), stB = av+den (2),
            # stC = qk+pv (2, closed mid-stream for the out-proj pools) ----
            stA = ExitStack()
            st_ps = stA.enter_context(
                tc.tile_pool(name="st_ps", bufs=2, space="PSUM"))   # 4 banks
            stB = ExitStack()
            av_ps = stB.enter_context(
                tc.tile_pool(name="av_ps", bufs=1, space="PSUM"))   # 1 bank
            den_ps = stB.enter_context(
                tc.tile_pool(name="den_ps", bufs=1, space="PSUM"))  # 1 bank
            stC = ExitStack()
            qk_ps = stC.enter_context(
                tc.tile_pool(name="qk_ps", bufs=1, space="PSUM"))   # 1 bank
            pv_ps = stC.enter_context(
                tc.tile_pool(name="pv_ps", bufs=1, space="PSUM"))   # 1 bank

            # ---- emission helpers (emission order == engine order) ----
            slabs = {}
            qk_pending = {}

            def load(kind, view, j):
                pool = {"q": qslab, "k": kslab, "v": vslab}[kind]
                t = pool.tile([P, SC, 512], F16, tag=kind, name="slab_t")
                nc.sync.dma_start(t[:], view[:, :, j * 512:(j + 1) * 512])
                slabs[(kind, j)] = t

            def proj_qk(kind, j, m, phase=2):
                """m-chunk (128 d_out) of Q/K projection for slab j.
                phase 0/1: half of the 8-matmul accumulation chain (so a
                hook steals at most ~0.9us of PE between score tiles);
                phase 2: whole chain."""
                t = slabs[(kind, j)]
                w_sb, b_sb, dst = ((wq_sb, bq_sb, QT) if kind == "q"
                                   else (wk_sb, bk_sb, KT))
                if phase in (0, 2):
                    ps = qk_ps.tile([P, 512], F32, tag="qk", name="ps")
                    qk_pending[(kind, j, m)] = ps
                else:
                    ps = qk_pending.pop((kind, j, m))
                kcs = {0: range(0, 4), 1: range(4, 8), 2: range(SC)}[phase]
                for kc in kcs:
                    nc.tensor.matmul(
                        ps[:], w_sb[:, kc, m * P:(m + 1) * P], t[:, kc, :],
                        start=(kc == 0), stop=(kc == SC - 1))
                if phase in (1, 2):
                    nc.vector.tensor_scalar_add(
                        dst[:, m, j * 512:(j + 1) * 512], ps[:],
                        b_sb[:, m:m + 1])

            def proj_v(c, h):
                """V2[:, c, h, :]: head h's V columns for k-chunk c."""
                t = slabs[("v", c // 4)]
                ss = c % 4
                psv = pv_ps.tile([P, HD], F32, tag="pv", name="psv")
                for kc in range(SC):
                    nc.tensor.matmul(
                        psv[:], t[:, kc, ss * P:(ss + 1) * P],
                        wv_sb[:, kc, h * HD:(h + 1) * HD],
                        start=(kc == 0), stop=(kc == SC - 1))
                nc.vector.tensor_copy(V2[:, c, h, 0:HD], psv[:])

            op_pools = [None, None]   # mid-stream + tail out-proj pools

            def out_proj(j, dve_only=False):
                """Output-projection chunk for query rows [128j, ..+128)."""
                pool = op_pools[j % 2] or op_pools[0]
                op = pool.tile([P, 1024], F32, tag="op", name="op")
                for n in range(2):
                    for ci in range(2):
                        nc.tensor.matmul(
                            op[:, n * 512:(n + 1) * 512],
                            xT[:, ci, j * P:(j + 1) * P],
                            wo_sb[:, ci, n * 512:(n + 1) * 512],
                            start=(ci == 0), stop=(ci == 1))
                g, c = j // 4, j % 4
                osb = osb_tiles[g % 2]
                if dve_only or j % 2 == 1:
                    nc.vector.tensor_copy(osb[:, c, :], op[:])
                else:
                    nc.scalar.copy(osb[:, c, :], op[:])
                if c == 3:
                    nc.sync.dma_start(outv[g], osb[:])

            def transpose(pair, qc):
                nc.sync.dma_start_transpose(
                    xT[:, pair, qc * P:(qc + 1) * P], xq[pair][:, qc, :])

            def attention(h, qh, hooks):
                """One (head, q-half) pass; hooks[kc] emits deferred work."""
                hp, hm = (h % 2) * HD, h // 2
                pair = h // 2
                av = av_ps.tile([P, 8, HD], F32, tag="av", name="av")
                den = den_ps.tile([P, 8, 1], F32, tag="den", name="den")

                def emit_av(kc, pt):
                    # One start/stop per PSUM bank: start pends the whole
                    # 2KB zero region; later qc first-writes zero-fill it.
                    for qc in range(8):
                        pc = pt[:, qc * P:(qc + 1) * P]
                        nc.tensor.matmul(
                            av[:, qc, :], pc, V2[:, kc, h, 0:HD],
                            start=(kc == 0 and qc == 0),
                            stop=(kc == KC - 1 and qc == 7),
                            skip_group_check=True)
                        nc.tensor.matmul(
                            den[:, qc, :], pc, V2[:, kc, h, HD:HD + 1],
                            start=(kc == 0 and qc == 0),
                            stop=(kc == KC - 1 and qc == 7),
                            skip_group_check=True)

                pending = None  # one-deep pipeline: PE never waits on ACT
                for kc in range(KC):
                    for hook in hooks.get(kc, ()):
                        hook()
                    st = st_ps.tile([P, 1024], F32, tag="st", name="st")
                    for qq in range(2):
                        q0 = qh * 1024 + qq * 512
                        nc.tensor.matmul(
                            st[:, qq * 512:(qq + 1) * 512],
                            KT[hp:hp + HD, hm, kc * P:(kc + 1) * P],
                            QT[hp:hp + HD, hm, q0:q0 + 512],
                            start=True, stop=True)
                    pt = ptp.tile([P, 1024], F16, tag="pt", name="pt")
                    nc.scalar.activation(
                        pt[:], st[:],
                        mybir.ActivationFunctionType.Exp, scale=0.125)
                    if pending is not None:
                        emit_av(*pending)
                    pending = (kc, pt)
                for hook in hooks.get(KC, ()):
                    hook()
                emit_av(*pending)

                rec = recp.tile([P, 8], F32, tag="rec", name="rec")
                nc.vector.reciprocal(rec[:], den[:, :, 0])
                nc.vector.tensor_tensor(
                    xq[pair][:, qh * 8:(qh + 1) * 8, hp:hp + HD],
                    av[:],
                    rec[:, :, None].to_broadcast([P, 8, HD]),
                    mybir.AluOpType.mult)

            # ---- prologue: DMAs in priority order, pipelined first projs --
            nc.sync.dma_start(wq_sb[:], wqT.rearrange("(kc p) m -> p kc m", p=P))
            nc.sync.dma_start(bq_sb[:], bq2.rearrange("m p -> p m"))
            nc.sync.dma_start(bk_sb[:], bk2.rearrange("m p -> p m"))
            load("q", qTv, 0)
            proj_qk("q", 0, 0)
            nc.sync.dma_start(wk_sb[:], wkT.rearrange("(kc p) m -> p kc m", p=P))
            load("k", kTv, 0)
            proj_qk("k", 0, 0)
            load("q", qTv, 1)
            proj_qk("q", 1, 0)
            nc.sync.dma_start(wv_sb[:], wvT.rearrange("(kc p) m -> p kc m", p=P))
            load("v", vTv, 0)
            load("k", kTv, 1)
            load("v", vTv, 1)
            load("k", kTv, 2)
            load("v", vTv, 2)
            load("k", kTv, 3)
            load("v", vTv, 3)
            nc.sync.dma_start(wo_sb[:], woT.rearrange("(c p) n -> p c n", p=P))
            load("q", qTv, 2)
            load("q", qTv, 3)

            def H(kind, j, m, phase):
                return lambda: proj_qk(kind, j, m, phase)

            # ---- attention sections (qh outer) with balanced hooks ----
            # sec0 (qh0,h0): k m0 half-chains just-in-time + h0's V chunks.
            hooks = {3: [H("k", 1, 0, 0)], 4: [H("k", 1, 0, 1)],
                     7: [H("k", 2, 0, 0)], 8: [H("k", 2, 0, 1)],
                     11: [H("k", 3, 0, 0)], 12: [H("k", 3, 0, 1)]}
            for c in range(KC):
                hooks.setdefault(c + 1, []).append(lambda c=c: proj_v(c, 0))
            attention(0, 0, hooks)

            # sec1 (qh0,h1): h1's V + q0/q1 m1 + k0 m1 (all data resident).
            hooks = {1: [H("q", 0, 1, 0)], 3: [H("q", 0, 1, 1)],
                     5: [H("q", 1, 1, 0)], 7: [H("q", 1, 1, 1)],
                     9: [H("k", 0, 1, 0)], 11: [H("k", 0, 1, 1)]}
            for c in range(KC):
                hooks.setdefault(c + 1, []).append(lambda c=c: proj_v(c, 1))
            attention(1, 0, hooks)
            for qc in range(8):
                transpose(0, qc)

            # sec2 (qh0,h2): h2's V + k1/k2/k3 m1 ahead of their columns.
            hooks = {0: [H("k", 1, 1, 0)], 2: [H("k", 1, 1, 1)],
                     5: [H("k", 2, 1, 0)], 7: [H("k", 2, 1, 1)],
                     9: [H("k", 3, 1, 0)], 11: [H("k", 3, 1, 1)]}
            for c in range(KC):
                hooks.setdefault(c + 1, []).append(lambda c=c: proj_v(c, 2))
            attention(2, 0, hooks)

            # sec3 (qh0,h3): h3's V + q2/q3 m0 (for qh1) + q2 m1.
            hooks = {3: [H("q", 2, 0, 0)], 5: [H("q", 2, 0, 1)],
                     7: [H("q", 3, 0, 0)], 9: [H("q", 3, 0, 1)],
                     11: [H("q", 2, 1, 0)], 13: [H("q", 2, 1, 1)]}
            for c in range(KC):
                hooks.setdefault(c + 1, []).append(lambda c=c: proj_v(c, 3))
            attention(3, 0, hooks)
            for qc in range(8):
                transpose(1, qc)

            # sec4 (qh1,h0): last projection (q3 m1), then close the qk/pv
            # banks and open the mid-stream out-proj pool.
            def open_op_mid():
                # right-side stack: av/den (left) close later, beneath it
                stC.close()
                op_pools[0] = op_pools[1] = tc.alloc_tile_pool(
                    name="op_mid", bufs=1, space="PSUM", side="right")
            hooks = {1: [H("q", 3, 1, 0)], 3: [H("q", 3, 1, 1)],
                     4: [open_op_mid],
                     8: [lambda: out_proj(0, True)],
                     12: [lambda: out_proj(1, True)]}
            attention(0, 1, hooks)
            attention(1, 1, {4: [lambda: out_proj(2, True)],
                             12: [lambda: out_proj(3, True)]})
            for qc in range(8, 16):
                transpose(0, qc)

            attention(2, 1, {4: [lambda: out_proj(4, True)],
                             12: [lambda: out_proj(5, True)]})
            attention(3, 1, {4: [lambda: out_proj(6, True)],
                             12: [lambda: out_proj(7, True)]})

            # ---- tail: free av/den, double-buffer out-proj via 2 pools,
            # interleave the final transposes ----
            stB.close()
            op_pools[1] = tc.alloc_tile_pool(name="op_tail", bufs=1,
                                             space="PSUM", side="right")
            for j in range(8, 16):
                transpose(1, j)
                out_proj(j)
            op_pools[1].release()
            op_pools[0].release()
            stA.close()

    nc.compile()
    return nc


_NC = None


def _get_nc():
    global _NC
    if _NC is None:
        _NC = _build_module()
    return _NC


def kernel(query, key, value, mask, Wq, bq, Wk, bk, Wv, bv, Wo, bo,
           _trace=False):
    query = np.asarray(query, np.float32)
    key = np.asarray(key, np.float32)
    value = np.asarray(value, np.float32)
    Wq, Wk, Wv, Wo = (np.asarray(w, np.float32) for w in (Wq, Wk, Wv, Wo))
    bq, bk, bv, bo = (np.asarray(b_, np.float32) for b_ in (bq, bk, bv, bo))
    mask = np.asarray(mask, bool)

    f16 = lambda x: np.ascontiguousarray(x, np.float16)
    qT = [f16(query[b].T) for b in range(B)]
    kTh = [f16(key[b].T) for b in range(B)]
    vTh = [f16(value[b].T) for b in range(B)]

    in_maps = []
    for c in range(NCORES):
        b, g = c // GROUPS, c % GROUPS
        gs = slice(g * DL, (g + 1) * DL)
        in_maps.append({
            "qT": qT[b], "kT": kTh[b], "vT": vTh[b],
            "wqT": f16(Wq[gs, :].T),
            "wkT": f16(Wk[gs, :].T),
            "wvT": f16(Wv[gs, :].T),
            "woT": f16(Wo[:, gs].T),
            "bq2": np.ascontiguousarray(bq[gs].reshape(2, P)),
            "bk2": np.ascontiguousarray(bk[gs].reshape(2, P)),
        })

    nc = _get_nc()
    res = run_bass_kernel_spmd(nc, in_maps, core_ids=list(range(NCORES)),
                               trace=_trace)

    extra = (bv @ Wo.T + bo).astype(np.float32)  # bv folds through out-proj
    output = np.zeros((B, S, D), np.float32)
    for c in range(NCORES):
        output[c // GROUPS] += res.results[c]["out"].astype(np.float32)
    output += extra

    # masked query rows attend uniformly (softmax of constant -1e9)
    if mask.any():
        for b in range(B):
            rows = np.nonzero(mask[b, 0])[0]
            if rows.size:
                v_full = value[b] @ Wv.T + bv
                out_row = v_full.mean(0) @ Wo.T + bo
                output[b, rows, :] = out_row

    if _trace:
        return output, res
    return output
